# revision 1
# baseline (speedup 1.0000x reference)
"""Trainium2 Bass kernel for nn_HSR_2_25116968747549 (gnn_message_passing).

The reference's edge construction (`tile(B,1).reshape(2,-1)`, the preserved
index-mixing bug) makes `edge_src == edge_dst` for every edge: all edges are
self-edges.  For a segment whose edges all share src == dst == n,
    out[n] = sum_e alpha_e * xl[src_e] = xl[n] * sum_e alpha_e = xl[n]
regardless of the attention logits, so each GATv2 layer collapses to the dense
affine map  x -> (x @ Wl + bl + cb) @ linw  and Wr/br/att never affect the
output.  The whole network is then

    t   = leaky_relu(x @ M1 + v1, 0.01)          M1 = Wl1@linw1@w1  (64x64)
    t_n = layernorm(t) * gamma + beta
    out = leaky_relu(t_n @ M2 + v2, 0.01)        M2 folded likewise

LayerNorm is folded further: (t - mu) = t @ C with C = I - J/64, and the
per-row rstd scale commutes past the second matmul, so on device we compute

    t    = leaky_relu(x @ M1 + v1)               (M1,v1 folded on host)
    a_r  = rsqrt(mean(t^2) - mean(t)^2 + eps)    (per row)
    out  = leaky_relu((a_r * t) @ M2c + v2)      M2c = C @ diag(gamma) @ M2

Sharding: data-parallel over windows; core c owns rows [1024c, 1024(c+1)).
Host passes x transposed+augmented ([65, 1024] feature-major with a ones row)
so the stationary matmul operand needs no on-chip transpose for layer 1; the
single mid-network transpose runs on the PE.
"""

import numpy as np

B, W, D, H = 256, 32, 64, 4
N = B * W
NCORES = 8
RPC = N // NCORES          # rows per core = 1024
TILES = RPC // 128         # 8 tiles of 128 rows
EPS = 1e-5
LRELU_SLOPE = 0.01


def _fold_weights(inp):
    f = lambda k: np.asarray(inp[k], np.float64)
    M1 = f("Wl1") @ f("linw1") @ f("w1")
    v1 = (f("bl1") + f("cb1")) @ f("linw1") @ f("w1") + f("b1")
    A2w = f("Wl2") @ f("linw2") @ f("w2")
    M2 = f("gamma")[:, None] * A2w
    v2 = f("beta") @ A2w + (f("bl2") + f("cb2")) @ f("linw2") @ f("w2") + f("b2")
    Cm = np.eye(D) - 1.0 / D
    M2c = Cm @ M2
    m1a = np.concatenate([M1, v1[None, :]], 0).astype(np.float32)  # [65, 64]
    m2a = np.concatenate([M2c, v2[None, :]], 0).astype(np.float32)  # [65, 64]
    return m1a, m2a


def _edges_degenerate(src, dst):
    src = np.asarray(src)
    dst = np.asarray(dst)
    return src.shape == dst.shape and np.array_equal(src, dst) and np.all(
        np.bincount(dst.astype(np.int64), minlength=N)[:N] > 0
    )


def _numpy_fallback(inp):
    # Generic (slow) host implementation, only used if the edge arrays ever
    # stop being fully degenerate.
    x = np.asarray(inp["x"], np.float32).reshape(N, D)
    src = np.asarray(inp["edge_src"]).astype(np.int64)
    dst = np.asarray(inp["edge_dst"]).astype(np.int64)

    def gat(xf, Wl, bl, Wr, br, att, cb, linw):
        xl = (xf @ Wl + bl).reshape(N, H, D)
        xr = (xf @ Wr + br).reshape(N, H, D)
        e = xl[src] + xr[dst]
        e = np.where(e > 0, e, 0.2 * e)
        logits = np.einsum("ehd,hd->eh", e, att)
        m = np.full((N, H), -np.inf, np.float32)
        np.maximum.at(m, dst, logits)
        ex = np.exp(logits - m[dst])
        den = np.zeros((N, H), np.float32)
        np.add.at(den, dst, ex)
        alpha = ex / den[dst]
        out = np.zeros((N, H, D), np.float32)
        np.add.at(out, dst, xl[src] * alpha[:, :, None])
        return (out.reshape(N, H * D) + cb) @ linw

    g = lambda k: np.asarray(inp[k], np.float32)
    lr = lambda t, a: np.where(t > 0, t, a * t)
    out = gat(x, g("Wl1"), g("bl1"), g("Wr1"), g("br1"), g("att1"), g("cb1"), g("linw1"))
    out = lr(out @ g("w1") + g("b1"), 0.01)
    mu = out.mean(-1, keepdims=True)
    var = ((out - mu) ** 2).mean(-1, keepdims=True)
    out = (out - mu) / np.sqrt(var + EPS) * g("gamma") + g("beta")
    out = gat(out, g("Wl2"), g("bl2"), g("Wr2"), g("br2"), g("att2"), g("cb2"), g("linw2"))
    out = lr(out @ g("w2") + g("b2"), 0.01)
    return out.reshape(B, W, D).astype(np.float32)


def build_bass():
    from concourse import bacc, mybir
    import concourse.tile as tile
    from concourse.masks import make_identity

    fp32 = mybir.dt.float32
    Act = mybir.ActivationFunctionType
    Alu = mybir.AluOpType

    nc = bacc.Bacc()
    xat_d = nc.declare_dram_parameter("xat", [D + 1, RPC], fp32, isOutput=False)
    m1_d = nc.declare_dram_parameter("m1a", [D + 1, D], fp32, isOutput=False)
    m2_d = nc.declare_dram_parameter("m2a", [D + 1, D], fp32, isOutput=False)
    y_d = nc.declare_dram_parameter("y", [RPC, D], fp32, isOutput=True)

    with tile.TileContext(nc) as tc:
        with (
            tc.tile_pool(name="const", bufs=1) as cpool,
            tc.tile_pool(name="psum", bufs=1, space="PSUM") as ppool,
            tc.tile_pool(name="work", bufs=3) as wpool,
        ):
            # ---- constants / persistent tiles ----
            ident = cpool.tile([128, 128], fp32, tag="ident")
            make_identity(nc, ident[:])
            xat = cpool.tile([D + 1, RPC], fp32, tag="xat")
            m1 = cpool.tile([D + 1, D], fp32, tag="m1")
            m2 = cpool.tile([D + 1, D], fp32, tag="m2")
            t_all = cpool.tile([128, TILES * D], fp32, tag="t_all")
            s1 = cpool.tile([128, TILES], fp32, tag="s1")
            s2 = cpool.tile([128, TILES], fp32, tag="s2")
            stats = cpool.tile([128, 4 * TILES], fp32, tag="stats")
            epsb = cpool.tile([128, 1], fp32, tag="epsb")
            nc.vector.memset(epsb[:], EPS)
            ones_row = cpool.tile([1, 128], fp32, tag="ones_row")
            nc.vector.memset(ones_row[:], 1.0)
            warm = cpool.tile([1, 1], fp32, tag="warm")
            # persistent PSUM tiles: disjoint column slices per row-tile, so
            # there is no slot recycling and no cross-engine release waits on
            # PE matmuls (HW allows one sync-wait per LDWEIGHTS slot).
            p1big = ppool.tile([128, TILES * D], fp32, tag="p1big")
            p2big = ppool.tile([128, TILES * D], fp32, tag="p2big")
            pTbig = ppool.tile([D, TILES * 128], fp32, tag="pTbig")
            wp = ppool.tile([D, 1], fp32, tag="wp")

            # ACT table warm-up: force the sqrt_and_others set (which also
            # contains leaky_relu/square/copy) to load while input DMA runs.
            nc.vector.memset(warm[:], 1.0)
            nc.scalar.activation(out=warm[:], in_=warm[:], func=Act.Sqrt)

            # ---- weight + input DMA ----
            v2row = cpool.tile([1, D], fp32, tag="v2row")
            nc.sync.dma_start(out=m1[:], in_=m1_d[:])
            nc.sync.dma_start(out=m2[:], in_=m2_d[:])
            nc.sync.dma_start(out=v2row[:], in_=m2_d[D:D + 1, :])
            NCHUNK = 4
            cw = RPC // NCHUNK
            for c in range(NCHUNK):
                nc.sync.dma_start(
                    out=xat[:, c * cw:(c + 1) * cw], in_=xat_d[:, c * cw:(c + 1) * cw]
                )

            # PE pre-consume of each weight DMA (one accumulation group):
            # the PE observes each DMA semaphore here, so the real matmuls
            # below need at most one wait each.
            nc.tensor.matmul(out=wp[:], lhsT=m1[0:D + 1, 0:D], rhs=m1[:, 0:1],
                             start=True, stop=False)
            nc.tensor.matmul(out=wp[:], lhsT=m2[0:D + 1, 0:D], rhs=m2[:, 0:1],
                             start=False, stop=False)
            nc.tensor.matmul(out=wp[:], lhsT=v2row[:], rhs=v2row[:, 0:1],
                             start=False, stop=True)

            # ---- phase A: t = lrelu(x @ M1 + v1), accumulate row stats ----
            for i in range(TILES):
                p1 = p1big[:, i * D:(i + 1) * D]
                nc.tensor.matmul(
                    out=p1,
                    lhsT=xat[:, i * 128:(i + 1) * 128],
                    rhs=m1[:],
                    start=True,
                    stop=True,
                )
                tsl = t_all[:, i * D:(i + 1) * D]
                # leaky_relu(x) = max(0.01*x, x), exact; two ops since only
                # one non-scalar PSUM read is allowed per instruction.
                lp = wpool.tile([128, D], fp32, tag="lp")
                nc.vector.tensor_scalar(
                    out=lp[:], in0=p1, scalar1=LRELU_SLOPE, scalar2=None,
                    op0=Alu.mult,
                )
                nc.vector.scalar_tensor_tensor(
                    out=tsl, in0=lp[:], scalar=1.0, in1=p1,
                    op0=Alu.mult, op1=Alu.max, accum_out=s1[:, i:i + 1],
                )
                sq = wpool.tile([128, D], fp32, tag="sq")
                nc.scalar.activation(
                    out=sq[:], in_=tsl, func=Act.Square, accum_out=s2[:, i:i + 1]
                )

            # ---- phase B: per-row scale a = rsqrt(var + eps), batched ----
            u = stats[:, 0:TILES]
            msq = stats[:, TILES:2 * TILES]
            var = stats[:, 2 * TILES:3 * TILES]
            a_all = stats[:, 3 * TILES:4 * TILES]
            nc.vector.tensor_scalar(
                out=u, in0=s1[:], scalar1=1.0 / D, scalar2=None, op0=Alu.mult
            )
            nc.vector.tensor_tensor(out=msq, in0=u, in1=u, op=Alu.mult)
            nc.vector.scalar_tensor_tensor(
                out=var, in0=s2[:], scalar=1.0 / D, in1=msq,
                op0=Alu.mult, op1=Alu.subtract,
            )
            sd = wpool.tile([128, TILES], fp32, tag="sd")
            nc.scalar.activation(out=sd[:], in_=var, func=Act.Sqrt, bias=epsb[:])
            nc.vector.reciprocal(out=a_all, in_=sd[:])

            # ---- phase C: out = lrelu((a*t) @ M2c + v2) ----
            for i in range(TILES):
                ta = wpool.tile([128, D], fp32, tag="ta")
                nc.vector.tensor_scalar(
                    out=ta[:], in0=t_all[:, i * D:(i + 1) * D],
                    scalar1=a_all[:, i:i + 1], scalar2=None, op0=Alu.mult,
                )
                pT = pTbig[:, i * 128:(i + 1) * 128]
                nc.tensor.transpose(out=pT, in_=ta[:], identity=ident[:])
                taT = wpool.tile([D, 128], fp32, tag="taT")
                nc.vector.tensor_copy(out=taT[:], in_=pT)
                p2 = p2big[:, i * D:(i + 1) * D]
                nc.tensor.matmul(
                    out=p2, lhsT=taT[:], rhs=m2[0:D, :], start=True, stop=False
                )
                # + ones(128) x v2 : bias add via PSUM accumulation
                nc.tensor.matmul(
                    out=p2, lhsT=ones_row[:], rhs=v2row[:],
                    start=False, stop=True,
                )
                lp2 = wpool.tile([128, D], fp32, tag="lp2")
                nc.vector.tensor_scalar(
                    out=lp2[:], in0=p2, scalar1=LRELU_SLOPE, scalar2=None,
                    op0=Alu.mult,
                )
                o = wpool.tile([128, D], fp32, tag="o")
                nc.vector.tensor_tensor(
                    out=o[:], in0=lp2[:], in1=p2, op=Alu.max,
                )
                nc.sync.dma_start(out=y_d[i * 128:(i + 1) * 128, :], in_=o[:])

    return nc


def kernel(**inputs):
    if not _edges_degenerate(inputs["edge_src"], inputs["edge_dst"]):
        return _numpy_fallback(inputs)

    from concourse.bass_utils import run_bass_kernel_spmd

    m1a, m2a = _fold_weights(inputs)
    xf = np.ascontiguousarray(np.asarray(inputs["x"], np.float32).reshape(N, D))
    ones = np.ones((RPC, 1), np.float32)
    in_maps = []
    for c in range(NCORES):
        xs = xf[c * RPC:(c + 1) * RPC]
        xat = np.ascontiguousarray(np.concatenate([xs, ones], 1).T)  # [65, 1024]
        in_maps.append({"xat": xat, "m1a": m1a, "m2a": m2a})

    nc = build_bass()
    if not nc.is_finalized():
        nc.finalize()
    res = run_bass_kernel_spmd(nc, in_maps, list(range(NCORES)))
    global LAST_RESULT
    LAST_RESULT = res
    out = np.concatenate([r["y"] for r in res.results], 0)
    return out.reshape(B, W, D).astype(np.float32)


LAST_RESULT = None


if __name__ == "__main__":
    x = np.random.randn(B, W, D).astype(np.float32)
    print("kernel module ok")



# revision 4
# speedup vs baseline: 1.6820x; 1.6820x over previous
"""Trainium2 Bass kernel for nn_HSR_2_25116968747549 (gnn_message_passing).

The reference's edge construction (`tile(B,1).reshape(2,-1)`, the preserved
index-mixing bug) makes `edge_src == edge_dst` for every edge: all edges are
self-edges.  For a segment whose edges all share src == dst == n,
    out[n] = sum_e alpha_e * xl[src_e] = xl[n] * sum_e alpha_e = xl[n]
regardless of the attention logits, so each GATv2 layer collapses to the dense
affine map  x -> (x @ Wl + bl + cb) @ linw  and Wr/br/att never affect the
output.  The whole network is then

    t   = leaky_relu(x @ M1 + v1, 0.01)          M1 = Wl1@linw1@w1  (64x64)
    t_n = layernorm(t) * gamma + beta
    out = leaky_relu(t_n @ M2 + v2, 0.01)        M2 folded likewise

LayerNorm folds further: (t - mu) = t @ C with C = I - J/64, and the per-row
rstd scale commutes past the second matmul, so on device:

    t    = lrelu(x @ M1 + v1)
    a_r  = rsqrt(mean(t^2) - mean(t)^2 + eps)
    out  = lrelu((a_r * t) @ M2c + v2)           M2c = C @ diag(gamma) @ M2

Device layout (per core, 1024 nodes): "two-half transposed" [128, 512]:
partitions 0-63 hold the 64 features of nodes 0-511 (one node per column),
partitions 64-127 hold nodes 512-1023.  This makes every matmul
weight-stationary with zero on-chip transposes:

  MM1:   psum1 = blockdiag(M1,M1)^T @ xT2            [128, 512]
  lrelu: t = ACT(Lrelu, psum1, bias=[v1;v1], alpha=0.01)  (scalar engine)
  MM2:   mean/meansq replicated per half via blockdiag(J/64,J/64) stationary
  stats: var = msq - mean^2; sd = sqrt(var+eps); rstd = 1/sd; ts = t*rstd
  MM3:   out tile i = (ts slice [64,128])^T @ M2c + ones^T v2   [128, 64]
  ACT(Lrelu) -> o_all -> 4 rearranged DMAs to row-major y.

All matmuls run in bf16 (1 cycle/row vs 4 for fp32; tolerance is 2e-2 and
bf16 lands ~1e-3).  Weights fold on host in fp64.
"""

import numpy as np

B, W, D, H = 256, 32, 64, 4
N = B * W
NCORES = 8
RPC = N // NCORES          # rows (nodes) per core = 1024
HALF = RPC // 2            # 512 nodes per half
EPS = 1e-5
SLOPE = 0.01


def _fold_weights(inp):
    f = lambda k: np.asarray(inp[k], np.float64)
    M1 = f("Wl1") @ f("linw1") @ f("w1")
    v1 = (f("bl1") + f("cb1")) @ f("linw1") @ f("w1") + f("b1")
    A2w = f("Wl2") @ f("linw2") @ f("w2")
    M2 = f("gamma")[:, None] * A2w
    v2 = f("beta") @ A2w + (f("bl2") + f("cb2")) @ f("linw2") @ f("w2") + f("b2")
    Cm = np.eye(D) - 1.0 / D
    M2c = Cm @ M2
    return M1, v1, M2c, v2


def _edges_degenerate(src, dst):
    src = np.asarray(src)
    dst = np.asarray(dst)
    return src.shape == dst.shape and np.array_equal(src, dst) and np.all(
        np.bincount(dst.astype(np.int64), minlength=N)[:N] > 0
    )


def _numpy_fallback(inp):
    # Generic (slow) host implementation, only used if the edge arrays ever
    # stop being fully degenerate.
    x = np.asarray(inp["x"], np.float32).reshape(N, D)
    src = np.asarray(inp["edge_src"]).astype(np.int64)
    dst = np.asarray(inp["edge_dst"]).astype(np.int64)

    def gat(xf, Wl, bl, Wr, br, att, cb, linw):
        xl = (xf @ Wl + bl).reshape(N, H, D)
        xr = (xf @ Wr + br).reshape(N, H, D)
        e = xl[src] + xr[dst]
        e = np.where(e > 0, e, 0.2 * e)
        logits = np.einsum("ehd,hd->eh", e, att)
        m = np.full((N, H), -np.inf, np.float32)
        np.maximum.at(m, dst, logits)
        ex = np.exp(logits - m[dst])
        den = np.zeros((N, H), np.float32)
        np.add.at(den, dst, ex)
        alpha = ex / den[dst]
        out = np.zeros((N, H, D), np.float32)
        np.add.at(out, dst, xl[src] * alpha[:, :, None])
        return (out.reshape(N, H * D) + cb) @ linw

    g = lambda k: np.asarray(inp[k], np.float32)
    lr = lambda t, a: np.where(t > 0, t, a * t)
    out = gat(x, g("Wl1"), g("bl1"), g("Wr1"), g("br1"), g("att1"), g("cb1"), g("linw1"))
    out = lr(out @ g("w1") + g("b1"), 0.01)
    mu = out.mean(-1, keepdims=True)
    var = ((out - mu) ** 2).mean(-1, keepdims=True)
    out = (out - mu) / np.sqrt(var + EPS) * g("gamma") + g("beta")
    out = gat(out, g("Wl2"), g("bl2"), g("Wr2"), g("br2"), g("att2"), g("cb2"), g("linw2"))
    out = lr(out @ g("w2") + g("b2"), 0.01)
    return out.reshape(B, W, D).astype(np.float32)


def build_bass():
    from concourse import bacc, mybir
    import concourse.tile as tile

    fp32 = mybir.dt.float32
    bf16 = mybir.dt.bfloat16
    Act = mybir.ActivationFunctionType
    Alu = mybir.AluOpType

    nc = bacc.Bacc()
    xat_d = nc.declare_dram_parameter("xat", [128, HALF], bf16, isOutput=False)
    wpk_d = nc.declare_dram_parameter("wpk", [128, 256], bf16, isOutput=False)
    v1b_d = nc.declare_dram_parameter("v1b", [128, 1], fp32, isOutput=False)
    y_d = nc.declare_dram_parameter("y", [RPC, D], fp32, isOutput=True)

    with tile.TileContext(nc) as tc:
        with (
            tc.tile_pool(name="const", bufs=1) as cpool,
            tc.tile_pool(name="psum", bufs=1, space="PSUM") as ppool,
            tc.tile_pool(name="pc", bufs=4, space="PSUM") as pcpool,
        ):
            # ---- persistent tiles ----
            xat = cpool.tile([128, HALF], bf16, tag="xat")
            wpk = cpool.tile([128, 256], bf16, tag="wpk")
            v1b = cpool.tile([128, 1], fp32, tag="v1b")
            sel = cpool.tile([128, 128], bf16, tag="sel")
            ones2 = cpool.tile([128, 128], bf16, tag="ones2")
            epsb = cpool.tile([128, 1], fp32, tag="epsb")
            warm = cpool.tile([1, 1], fp32, tag="warm")
            t_sb = cpool.tile([128, HALF], bf16, tag="t_sb")
            sq_sb = cpool.tile([128, HALF], bf16, tag="sq_sb")
            mean2 = cpool.tile([128, HALF], fp32, tag="mean2")
            var_sb = cpool.tile([128, HALF], fp32, tag="var_sb")
            sd_sb = cpool.tile([128, HALF], fp32, tag="sd_sb")
            rstd = cpool.tile([128, HALF], fp32, tag="rstd")
            ts_sb = cpool.tile([128, HALF], bf16, tag="ts_sb")
            o_all = cpool.tile([128, HALF], fp32, tag="o_all")

            psum1 = ppool.tile([128, HALF], fp32, tag="psum1")
            pmean = ppool.tile([128, HALF], fp32, tag="pmean")
            pmsq = ppool.tile([128, HALF], fp32, tag="pmsq")

            wblk = wpk[:, 0:128]          # blockdiag(M1, M1)
            m2two = wpk[:, 128:192]       # M2c stacked twice (rows 0-63 / 64-127)
            v2two = wpk[:, 192:256]       # v2 in rows 0 and 64

            # ---- input DMAs + constants (first thing on each queue) ----
            nc.sync.dma_start(out=xat[:], in_=xat_d[:])
            nc.sync.dma_start(out=v1b[:], in_=v1b_d[:])
            nc.gpsimd.dma_start(out=wpk[:], in_=wpk_d[:])
            # ACT table warm-up: Lrelu/Square/Sqrt all live in the sqrt set;
            # loading it here overlaps the input DMAs.
            nc.vector.memset(warm[:], 1.0)
            nc.scalar.activation(out=warm[:], in_=warm[:], func=Act.Sqrt)
            # selector blockdiag(J/64, J/64) and the ones rows, built on chip
            nc.gpsimd.memset(sel[:], 0.0)
            nc.gpsimd.memset(sel[0:64, 0:64], 1.0 / D)
            nc.gpsimd.memset(sel[64:128, 64:128], 1.0 / D)
            nc.gpsimd.memset(ones2[:], 1.0)
            nc.gpsimd.memset(epsb[:], EPS)

            NCH = 2
            CW = HALF // NCH  # 256-column chunks
            # ---- MM1 + lrelu + square, chunked to overlap engines ----
            for c in range(NCH):
                cs = slice(c * CW, (c + 1) * CW)
                nc.tensor.matmul(
                    out=psum1[:, cs], lhsT=wblk, rhs=xat[:, cs],
                    start=True, stop=True,
                )
            for c in range(NCH):
                cs = slice(c * CW, (c + 1) * CW)
                # t = lrelu(pre + v1), one scalar-engine op (alpha = slope)
                nc.scalar.activation(
                    out=t_sb[:, cs], in_=psum1[:, cs], func=Act.Lrelu,
                    bias=v1b[:], alpha=SLOPE,
                )
                nc.vector.tensor_tensor(
                    out=sq_sb[:, cs], in0=t_sb[:, cs], in1=t_sb[:, cs],
                    op=Alu.mult,
                )
                nc.tensor.matmul(
                    out=pmean[:, cs], lhsT=sel[:], rhs=t_sb[:, cs],
                    start=True, stop=True,
                )
                nc.tensor.matmul(
                    out=pmsq[:, cs], lhsT=sel[:], rhs=sq_sb[:, cs],
                    start=True, stop=True,
                )

            # ---- stats: rstd = 1/sqrt(E[t^2] - E[t]^2 + eps), replicated ----
            for c in range(NCH):
                cs = slice(c * CW, (c + 1) * CW)
                nc.scalar.activation(
                    out=mean2[:, cs], in_=pmean[:, cs], func=Act.Square,
                )
                nc.vector.scalar_tensor_tensor(
                    out=var_sb[:, cs], in0=pmsq[:, cs], scalar=1.0,
                    in1=mean2[:, cs], op0=Alu.mult, op1=Alu.subtract,
                )
                nc.scalar.activation(
                    out=sd_sb[:, cs], in_=var_sb[:, cs], func=Act.Sqrt,
                    bias=epsb[:],
                )
                nc.vector.reciprocal(out=rstd[:, cs], in_=sd_sb[:, cs])
                nc.vector.tensor_tensor(
                    out=ts_sb[:, cs], in0=t_sb[:, cs], in1=rstd[:, cs],
                    op=Alu.mult,
                )

            # ---- phase C: per 128-node tile, (ts slice)^T @ M2c + v2 ----
            # chunk c=0 covers tiles {0,1} (head) and {4,5} (tail).
            order = [0, 1, 4, 5, 2, 3, 6, 7]
            for i in order:
                h, j = divmod(i, 4)
                hp = slice(64 * h, 64 * h + 64)
                pc = pcpool.tile([128, D], fp32, tag="pc")
                nc.tensor.matmul(
                    out=pc[:],
                    lhsT=ts_sb[hp, 128 * j:128 * j + 128],
                    rhs=m2two[hp, :],
                    start=True, stop=False,
                )
                nc.tensor.matmul(
                    out=pc[:],
                    lhsT=ones2[64 * h:64 * h + 1, 0:128],
                    rhs=v2two[64 * h:64 * h + 1, :],
                    start=False, stop=True,
                )
                nc.scalar.activation(
                    out=o_all[:, 64 * i:64 * i + 64], in_=pc[:],
                    func=Act.Lrelu, alpha=SLOPE,
                )

            # ---- outputs: pairs of tiles, rearranged to row-major y ----
            for a, b in ((0, 1), (4, 5), (2, 3), (6, 7)):
                dst = y_d[128 * a:128 * a + 256, :]
                dst = dst.rearrange("(i p) f -> p i f", i=2, p=128)
                src = o_all[:, 64 * a:64 * a + 128].rearrange(
                    "p (i f) -> p i f", i=2, f=D
                )
                nc.sync.dma_start(out=dst, in_=src)

    return nc


def kernel(**inputs):
    if not _edges_degenerate(inputs["edge_src"], inputs["edge_dst"]):
        return _numpy_fallback(inputs)

    import ml_dtypes
    from concourse.bass_utils import run_bass_kernel_spmd

    bf = ml_dtypes.bfloat16
    M1, v1, M2c, v2 = _fold_weights(inputs)

    wblk = np.zeros((128, 128), np.float64)
    wblk[0:64, 0:64] = M1
    wblk[64:128, 64:128] = M1
    m2two = np.vstack([M2c, M2c])                     # [128, 64]
    v2two = np.zeros((128, 64), np.float64)
    v2two[0, :] = v2
    v2two[64, :] = v2
    wpk = np.hstack([wblk, m2two, v2two]).astype(bf)  # [128, 256]
    v1b = np.concatenate([v1, v1]).reshape(128, 1).astype(np.float32)

    xf = np.asarray(inputs["x"], np.float32).reshape(N, D)
    in_maps = []
    for c in range(NCORES):
        xs = xf[c * RPC:(c + 1) * RPC]                # [1024, 64]
        xat = np.concatenate(
            [xs[0:HALF].T, xs[HALF:RPC].T], axis=0    # [128, 512]
        ).astype(bf)
        in_maps.append({
            "xat": np.ascontiguousarray(xat),
            "wpk": wpk,
            "v1b": v1b,
        })

    nc = build_bass()
    if not nc.is_finalized():
        nc.finalize()
    res = run_bass_kernel_spmd(nc, in_maps, list(range(NCORES)))
    global LAST_RESULT
    LAST_RESULT = res
    out = np.concatenate([r["y"] for r in res.results], 0)
    return out.reshape(B, W, D).astype(np.float32)


LAST_RESULT = None


if __name__ == "__main__":
    print("kernel module ok")


# revision 7
# speedup vs baseline: 2.0809x; 1.2371x over previous
"""Trainium2 Bass kernel for nn_HSR_2_25116968747549 (gnn_message_passing).

The reference's edge construction (`tile(B,1).reshape(2,-1)`, the preserved
index-mixing bug) makes `edge_src == edge_dst` for every edge: all edges are
self-edges.  For a segment whose edges all share src == dst == n,
    out[n] = sum_e alpha_e * xl[src_e] = xl[n] * sum_e alpha_e = xl[n]
regardless of the attention logits, so each GATv2 layer collapses to the dense
affine map  x -> (x @ Wl + bl + cb) @ linw  and Wr/br/att never affect the
output.  The whole network is then

    t   = leaky_relu(x @ M1 + v1, 0.01)          M1 = Wl1@linw1@w1  (64x64)
    t_n = layernorm(t) * gamma + beta
    out = leaky_relu(t_n @ M2 + v2, 0.01)        M2 folded likewise

LayerNorm folds further: (t - mu) = t @ C with C = I - J/64, and the per-row
rstd scale commutes past the second matmul, so on device:

    t    = lrelu(x @ M1 + v1)
    a_r  = rsqrt(mean(t^2) - mean(t)^2 + eps)
    out  = lrelu((a_r * t) @ M2c + v2)           M2c = C @ diag(gamma) @ M2

Device layout (per core, 1024 nodes): "two-half transposed" [128, 512]:
partitions 0-63 hold the 64 features of nodes 0-511 (one node per column),
partitions 64-127 hold nodes 512-1023.  This makes every matmul
weight-stationary with zero on-chip transposes:

  MM1:   psum1 = blockdiag(M1,M1)^T @ xT2            [128, 512]
  lrelu: t = ACT(Lrelu, psum1, bias=[v1;v1], alpha=0.01)  (scalar engine)
  MM2:   mean/meansq replicated per half via blockdiag(J/64,J/64) stationary
  stats: var = msq - mean^2; sd = sqrt(var+eps); rstd = 1/sd; ts = t*rstd
  MM3:   out tile i = (ts slice [64,128])^T @ M2c + ones^T v2   [128, 64]
  ACT(Lrelu) -> o_all -> 4 rearranged DMAs to row-major y.

All matmuls run in bf16 (1 cycle/row vs 4 for fp32; tolerance is 2e-2 and
bf16 lands ~1e-3).  Weights fold on host in fp64.
"""

import numpy as np

B, W, D, H = 256, 32, 64, 4
N = B * W
NCORES = 8
RPC = N // NCORES          # rows (nodes) per core = 1024
HALF = RPC // 2            # 512 nodes per half
EPS = 1e-5
SLOPE = 0.01


def _fold_weights(inp):
    f = lambda k: np.asarray(inp[k], np.float64)
    M1 = f("Wl1") @ f("linw1") @ f("w1")
    v1 = (f("bl1") + f("cb1")) @ f("linw1") @ f("w1") + f("b1")
    A2w = f("Wl2") @ f("linw2") @ f("w2")
    M2 = f("gamma")[:, None] * A2w
    v2 = f("beta") @ A2w + (f("bl2") + f("cb2")) @ f("linw2") @ f("w2") + f("b2")
    Cm = np.eye(D) - 1.0 / D
    M2c = Cm @ M2
    return M1, v1, M2c, v2


def _edges_degenerate(src, dst):
    src = np.asarray(src)
    dst = np.asarray(dst)
    return src.shape == dst.shape and np.array_equal(src, dst) and np.all(
        np.bincount(dst.astype(np.int64), minlength=N)[:N] > 0
    )


def _numpy_fallback(inp):
    # Generic (slow) host implementation, only used if the edge arrays ever
    # stop being fully degenerate.
    x = np.asarray(inp["x"], np.float32).reshape(N, D)
    src = np.asarray(inp["edge_src"]).astype(np.int64)
    dst = np.asarray(inp["edge_dst"]).astype(np.int64)

    def gat(xf, Wl, bl, Wr, br, att, cb, linw):
        xl = (xf @ Wl + bl).reshape(N, H, D)
        xr = (xf @ Wr + br).reshape(N, H, D)
        e = xl[src] + xr[dst]
        e = np.where(e > 0, e, 0.2 * e)
        logits = np.einsum("ehd,hd->eh", e, att)
        m = np.full((N, H), -np.inf, np.float32)
        np.maximum.at(m, dst, logits)
        ex = np.exp(logits - m[dst])
        den = np.zeros((N, H), np.float32)
        np.add.at(den, dst, ex)
        alpha = ex / den[dst]
        out = np.zeros((N, H, D), np.float32)
        np.add.at(out, dst, xl[src] * alpha[:, :, None])
        return (out.reshape(N, H * D) + cb) @ linw

    g = lambda k: np.asarray(inp[k], np.float32)
    lr = lambda t, a: np.where(t > 0, t, a * t)
    out = gat(x, g("Wl1"), g("bl1"), g("Wr1"), g("br1"), g("att1"), g("cb1"), g("linw1"))
    out = lr(out @ g("w1") + g("b1"), 0.01)
    mu = out.mean(-1, keepdims=True)
    var = ((out - mu) ** 2).mean(-1, keepdims=True)
    out = (out - mu) / np.sqrt(var + EPS) * g("gamma") + g("beta")
    out = gat(out, g("Wl2"), g("bl2"), g("Wr2"), g("br2"), g("att2"), g("cb2"), g("linw2"))
    out = lr(out @ g("w2") + g("b2"), 0.01)
    return out.reshape(B, W, D).astype(np.float32)


def build_bass():
    from concourse import bacc, mybir
    import concourse.tile as tile

    fp32 = mybir.dt.float32
    bf16 = mybir.dt.bfloat16
    Act = mybir.ActivationFunctionType
    Alu = mybir.AluOpType

    nc = bacc.Bacc()
    # cols 0-1: v1 (fp32 bitcast as 2 bf16 cols); cols 2-513: x two-half layout
    xat_d = nc.declare_dram_parameter("xat", [128, HALF + 2], bf16, isOutput=False)
    wpk_d = nc.declare_dram_parameter("wpk", [128, 256], bf16, isOutput=False)
    y_d = nc.declare_dram_parameter("y", [RPC, D], fp32, isOutput=True)

    with tile.TileContext(nc) as tc:
        with (
            tc.tile_pool(name="const", bufs=1) as cpool,
            tc.tile_pool(name="psum", bufs=1, space="PSUM") as ppool,
            tc.tile_pool(name="pc", bufs=4, space="PSUM") as pcpool,
        ):
            # ---- persistent tiles ----
            xat = cpool.tile([128, HALF + 2], bf16, tag="xat")
            wpk = cpool.tile([128, 256], bf16, tag="wpk")
            sel = cpool.tile([128, 128], bf16, tag="sel")
            ones2 = cpool.tile([128, 128], bf16, tag="ones2")
            epsb = cpool.tile([128, 1], fp32, tag="epsb")
            warm = cpool.tile([1, 1], fp32, tag="warm")
            t_sb = cpool.tile([128, HALF], bf16, tag="t_sb")
            sq_sb = cpool.tile([128, HALF], bf16, tag="sq_sb")
            mean2 = cpool.tile([128, HALF], fp32, tag="mean2")
            rstd = cpool.tile([128, HALF], fp32, tag="rstd")
            ts_sb = cpool.tile([128, HALF], bf16, tag="ts_sb")
            o_all = cpool.tile([128, HALF], fp32, tag="o_all")

            psum1 = ppool.tile([128, HALF], fp32, tag="psum1")
            pmean = ppool.tile([128, HALF], fp32, tag="pmean")
            pmsq = ppool.tile([128, HALF], fp32, tag="pmsq")

            wblk = wpk[:, 0:128]          # blockdiag(M1, M1)
            m2two = wpk[:, 128:192]       # M2c stacked twice (rows 0-63 / 64-127)
            v2two = wpk[:, 192:256]       # v2 in rows 0 and 64
            v1b = xat[:, 0:2].bitcast(fp32)  # [128, 1] fp32 view

            NCH = 2
            CW = HALF // NCH  # 256-column chunks

            def xcol(c):  # data columns of chunk c (skipping the v1 prefix)
                return xat[:, 2 + c * CW:2 + (c + 1) * CW]

            # ---- input DMAs + constants (first thing on each queue) ----
            for c in range(NCH):
                lo = 0 if c == 0 else 2 + c * CW
                hi = 2 + (c + 1) * CW
                nc.sync.dma_start(out=xat[:, lo:hi], in_=xat_d[:, lo:hi])
            nc.scalar.dma_start(out=wpk[:], in_=wpk_d[:])
            # ACT warm-up: Prelu/Square/Abs_reciprocal_sqrt all live in the
            # abs_reciprocal_sqrt_and_small set -> exactly one table load,
            # overlapped with the input DMAs.
            nc.vector.memset(warm[:], 1.0)
            nc.scalar.activation(
                out=warm[:], in_=warm[:], func=Act.Abs_reciprocal_sqrt
            )
            # selector blockdiag(J/64, J/64) and the ones rows, built on chip
            nc.gpsimd.memset(sel[:], 0.0)
            nc.gpsimd.memset(sel[0:64, 0:64], 1.0 / D)
            nc.gpsimd.memset(sel[64:128, 64:128], 1.0 / D)
            nc.gpsimd.memset(ones2[:], 1.0)
            nc.gpsimd.memset(epsb[:], EPS)

            # ---- MM1 + prelu + square + stats matmuls, chunked ----
            for c in range(NCH):
                cs = slice(c * CW, (c + 1) * CW)
                nc.tensor.matmul(
                    out=psum1[:, cs], lhsT=wblk, rhs=xcol(c),
                    start=True, stop=True,
                )
                # t = lrelu(pre + v1): parametric relu, alpha = slope
                nc.scalar.activation(
                    out=t_sb[:, cs], in_=psum1[:, cs], func=Act.Prelu,
                    bias=v1b, alpha=SLOPE,
                )
                nc.vector.tensor_tensor(
                    out=sq_sb[:, cs], in0=t_sb[:, cs], in1=t_sb[:, cs],
                    op=Alu.mult,
                )
                nc.tensor.matmul(
                    out=pmean[:, cs], lhsT=sel[:], rhs=t_sb[:, cs],
                    start=True, stop=True,
                )
                nc.tensor.matmul(
                    out=pmsq[:, cs], lhsT=sel[:], rhs=sq_sb[:, cs],
                    start=True, stop=True,
                )

            # ---- stats: rstd = 1/sqrt(E[t^2] - E[t]^2 + eps), replicated ----
            for c in range(NCH):
                cs = slice(c * CW, (c + 1) * CW)
                nc.scalar.activation(
                    out=mean2[:, cs], in_=pmean[:, cs], func=Act.Square,
                )
                nc.vector.scalar_tensor_tensor(
                    out=rstd[:, cs], in0=pmsq[:, cs], scalar=1.0,
                    in1=mean2[:, cs], op0=Alu.mult, op1=Alu.subtract,
                )
                # rstd = 1/sqrt(var + eps), single ACT op (var >= 0)
                nc.scalar.activation(
                    out=rstd[:, cs], in_=rstd[:, cs],
                    func=Act.Abs_reciprocal_sqrt, bias=epsb[:],
                )
                nc.vector.tensor_tensor(
                    out=ts_sb[:, cs], in0=t_sb[:, cs], in1=rstd[:, cs],
                    op=Alu.mult,
                )

            # ---- phase C: per 128-node tile, (ts slice)^T @ M2c + v2 ----
            # chunk c=0 covers tiles {0,1} (head) and {4,5} (tail).
            order = [0, 1, 4, 5, 2, 3, 6, 7]
            for i in order:
                h, j = divmod(i, 4)
                hp = slice(64 * h, 64 * h + 64)
                pc = pcpool.tile([128, D], fp32, tag="pc")
                nc.tensor.matmul(
                    out=pc[:],
                    lhsT=ts_sb[hp, 128 * j:128 * j + 128],
                    rhs=m2two[hp, :],
                    start=True, stop=False,
                )
                nc.tensor.matmul(
                    out=pc[:],
                    lhsT=ones2[64 * h:64 * h + 1, 0:128],
                    rhs=v2two[64 * h:64 * h + 1, :],
                    start=False, stop=True,
                )
                nc.scalar.activation(
                    out=o_all[:, 64 * i:64 * i + 64], in_=pc[:],
                    func=Act.Prelu, alpha=SLOPE,
                )

            # ---- outputs: pairs of tiles, rearranged to row-major y ----
            for a, b in ((0, 1), (4, 5), (2, 3), (6, 7)):
                dst = y_d[128 * a:128 * a + 256, :]
                dst = dst.rearrange("(i p) f -> p i f", i=2, p=128)
                src = o_all[:, 64 * a:64 * a + 128].rearrange(
                    "p (i f) -> p i f", i=2, f=D
                )
                nc.sync.dma_start(out=dst, in_=src)

    return nc


def kernel(**inputs):
    if not _edges_degenerate(inputs["edge_src"], inputs["edge_dst"]):
        return _numpy_fallback(inputs)

    import ml_dtypes
    from concourse.bass_utils import run_bass_kernel_spmd

    bf = ml_dtypes.bfloat16
    M1, v1, M2c, v2 = _fold_weights(inputs)

    wblk = np.zeros((128, 128), np.float64)
    wblk[0:64, 0:64] = M1
    wblk[64:128, 64:128] = M1
    m2two = np.vstack([M2c, M2c])                     # [128, 64]
    v2two = np.zeros((128, 64), np.float64)
    v2two[0, :] = v2
    v2two[64, :] = v2
    wpk = np.hstack([wblk, m2two, v2two]).astype(bf)  # [128, 256]
    # v1 duplicated per half, fp32, carried as 2 bf16 columns of xat
    v1b = np.concatenate([v1, v1]).reshape(128, 1).astype(np.float32)
    v1cols = v1b.view(np.uint16).view(bf)             # [128, 2]

    xf = np.asarray(inputs["x"], np.float32).reshape(N, D)
    in_maps = []
    for c in range(NCORES):
        xs = xf[c * RPC:(c + 1) * RPC]                # [1024, 64]
        xat = np.concatenate(
            [xs[0:HALF].T, xs[HALF:RPC].T], axis=0    # [128, 512]
        ).astype(bf)
        xat = np.concatenate([v1cols, xat], axis=1)   # [128, 514]
        in_maps.append({
            "xat": np.ascontiguousarray(xat),
            "wpk": wpk,
        })

    nc = build_bass()
    if not nc.is_finalized():
        nc.finalize()
    res = run_bass_kernel_spmd(nc, in_maps, list(range(NCORES)))
    global LAST_RESULT
    LAST_RESULT = res
    out = np.concatenate([r["y"] for r in res.results], 0)
    return out.reshape(B, W, D).astype(np.float32)


LAST_RESULT = None


if __name__ == "__main__":
    print("kernel module ok")


# revision 10
# speedup vs baseline: 2.1655x; 1.0407x over previous
"""Trainium2 Bass kernel for nn_HSR_2_25116968747549 (gnn_message_passing).

The reference's edge construction (`tile(B,1).reshape(2,-1)`, the preserved
index-mixing bug) makes `edge_src == edge_dst` for every edge: all edges are
self-edges.  For a segment whose edges all share src == dst == n,
    out[n] = sum_e alpha_e * xl[src_e] = xl[n] * sum_e alpha_e = xl[n]
regardless of the attention logits, so each GATv2 layer collapses to the dense
affine map  x -> (x @ Wl + bl + cb) @ linw  and Wr/br/att never affect the
output.  The whole network is then

    t   = leaky_relu(x @ M1 + v1, 0.01)          M1 = Wl1@linw1@w1  (64x64)
    t_n = layernorm(t) * gamma + beta
    out = leaky_relu(t_n @ M2 + v2, 0.01)        M2 folded likewise

LayerNorm folds further: (t - mu) = t @ C with C = I - J/64, and the per-row
rstd scale commutes past the second matmul, so on device:

    t    = lrelu(x @ M1 + v1)
    a_r  = rsqrt(mean(t^2) - mean(t)^2 + eps)
    out  = lrelu((a_r * t) @ M2c + v2)           M2c = C @ diag(gamma) @ M2

Device layout (per core, 1024 nodes): "two-half transposed" [128, 512]:
partitions 0-63 hold the 64 features of nodes 0-511 (one node per column),
partitions 64-127 hold nodes 512-1023.  This makes every matmul
weight-stationary with zero on-chip transposes:

  MM1:   psum1 = blockdiag(M1,M1)^T @ xT2            [128, 512]
  lrelu: t = ACT(Lrelu, psum1, bias=[v1;v1], alpha=0.01)  (scalar engine)
  MM2:   mean/meansq replicated per half via blockdiag(J/64,J/64) stationary
  stats: var = msq - mean^2; sd = sqrt(var+eps); rstd = 1/sd; ts = t*rstd
  MM3:   out tile i = (ts slice [64,128])^T @ M2c + ones^T v2   [128, 64]
  ACT(Lrelu) -> o_all -> 4 rearranged DMAs to row-major y.

All matmuls run in bf16 (1 cycle/row vs 4 for fp32; tolerance is 2e-2 and
bf16 lands ~1e-3).  Weights fold on host in fp64.
"""

import numpy as np

B, W, D, H = 256, 32, 64, 4
N = B * W
NCORES = 8
RPC = N // NCORES          # rows (nodes) per core = 1024
HALF = RPC // 2            # 512 nodes per half
EPS = 1e-5
SLOPE = 0.01


def _fold_weights(inp):
    f = lambda k: np.asarray(inp[k], np.float64)
    M1 = f("Wl1") @ f("linw1") @ f("w1")
    v1 = (f("bl1") + f("cb1")) @ f("linw1") @ f("w1") + f("b1")
    A2w = f("Wl2") @ f("linw2") @ f("w2")
    M2 = f("gamma")[:, None] * A2w
    v2 = f("beta") @ A2w + (f("bl2") + f("cb2")) @ f("linw2") @ f("w2") + f("b2")
    Cm = np.eye(D) - 1.0 / D
    M2c = Cm @ M2
    return M1, v1, M2c, v2


def _edges_degenerate(src, dst):
    src = np.asarray(src)
    dst = np.asarray(dst)
    return src.shape == dst.shape and np.array_equal(src, dst) and np.all(
        np.bincount(dst.astype(np.int64), minlength=N)[:N] > 0
    )


def _numpy_fallback(inp):
    # Generic (slow) host implementation, only used if the edge arrays ever
    # stop being fully degenerate.
    x = np.asarray(inp["x"], np.float32).reshape(N, D)
    src = np.asarray(inp["edge_src"]).astype(np.int64)
    dst = np.asarray(inp["edge_dst"]).astype(np.int64)

    def gat(xf, Wl, bl, Wr, br, att, cb, linw):
        xl = (xf @ Wl + bl).reshape(N, H, D)
        xr = (xf @ Wr + br).reshape(N, H, D)
        e = xl[src] + xr[dst]
        e = np.where(e > 0, e, 0.2 * e)
        logits = np.einsum("ehd,hd->eh", e, att)
        m = np.full((N, H), -np.inf, np.float32)
        np.maximum.at(m, dst, logits)
        ex = np.exp(logits - m[dst])
        den = np.zeros((N, H), np.float32)
        np.add.at(den, dst, ex)
        alpha = ex / den[dst]
        out = np.zeros((N, H, D), np.float32)
        np.add.at(out, dst, xl[src] * alpha[:, :, None])
        return (out.reshape(N, H * D) + cb) @ linw

    g = lambda k: np.asarray(inp[k], np.float32)
    lr = lambda t, a: np.where(t > 0, t, a * t)
    out = gat(x, g("Wl1"), g("bl1"), g("Wr1"), g("br1"), g("att1"), g("cb1"), g("linw1"))
    out = lr(out @ g("w1") + g("b1"), 0.01)
    mu = out.mean(-1, keepdims=True)
    var = ((out - mu) ** 2).mean(-1, keepdims=True)
    out = (out - mu) / np.sqrt(var + EPS) * g("gamma") + g("beta")
    out = gat(out, g("Wl2"), g("bl2"), g("Wr2"), g("br2"), g("att2"), g("cb2"), g("linw2"))
    out = lr(out @ g("w2") + g("b2"), 0.01)
    return out.reshape(B, W, D).astype(np.float32)


def build_bass():
    from concourse import bacc, mybir
    import concourse.tile as tile

    fp32 = mybir.dt.float32
    bf16 = mybir.dt.bfloat16
    Act = mybir.ActivationFunctionType
    Alu = mybir.AluOpType

    nc = bacc.Bacc()
    # cols 0-1: v1 (fp32 bitcast as 2 bf16 cols); cols 2-513: x two-half layout
    xat_d = nc.declare_dram_parameter("xat", [128, HALF + 2], bf16, isOutput=False)
    wpk_d = nc.declare_dram_parameter("wpk", [128, 256], bf16, isOutput=False)
    y_d = nc.declare_dram_parameter("y", [RPC, D], fp32, isOutput=True)

    with tile.TileContext(nc) as tc:
        with (
            tc.tile_pool(name="const", bufs=1) as cpool,
            tc.tile_pool(name="psum", bufs=1, space="PSUM") as ppool,
            tc.tile_pool(name="pc", bufs=2, space="PSUM") as pcpool,
        ):
            # ---- persistent tiles ----
            xat = cpool.tile([128, HALF + 2], bf16, tag="xat")
            wpk = cpool.tile([128, 256], bf16, tag="wpk")
            sel = cpool.tile([128, 128], bf16, tag="sel")
            ones2 = cpool.tile([128, 128], bf16, tag="ones2")
            epsb = cpool.tile([128, 1], fp32, tag="epsb")
            warm = cpool.tile([1, 1], fp32, tag="warm")
            t_sb = cpool.tile([128, HALF], bf16, tag="t_sb")
            sq_sb = cpool.tile([128, HALF], bf16, tag="sq_sb")
            mean2 = cpool.tile([128, HALF], fp32, tag="mean2")
            rstd = cpool.tile([128, HALF], fp32, tag="rstd")
            ts_sb = cpool.tile([128, HALF], bf16, tag="ts_sb")
            o_all = cpool.tile([128, HALF], fp32, tag="o_all")

            NCH = 2
            CW = HALF // NCH  # 256-column chunks
            psum1 = [ppool.tile([128, CW], fp32, name=f"psum1_{c}", tag=f"psum1_{c}") for c in range(NCH)]
            pmean = [ppool.tile([128, CW], fp32, name=f"pmean_{c}", tag=f"pmean_{c}") for c in range(NCH)]
            pmsq = [ppool.tile([128, CW], fp32, name=f"pmsq_{c}", tag=f"pmsq_{c}") for c in range(NCH)]

            wblk = wpk[:, 0:128]          # blockdiag(M1, M1)
            m2two = wpk[:, 128:192]       # M2c stacked twice (rows 0-63 / 64-127)
            v2two = wpk[:, 192:256]       # v2 in rows 0 and 64
            v1b = xat[:, 0:2].bitcast(fp32)  # [128, 1] fp32 view

            def xcol(c):  # data columns of chunk c (skipping the v1 prefix)
                return xat[:, 2 + c * CW:2 + (c + 1) * CW]

            # ---- input DMAs + constants (first thing on each queue) ----
            for c in range(NCH):
                lo = 0 if c == 0 else 2 + c * CW
                hi = 2 + (c + 1) * CW
                nc.sync.dma_start(out=xat[:, lo:hi], in_=xat_d[:, lo:hi])
            nc.scalar.dma_start(out=wpk[:], in_=wpk_d[:])
            # ACT warm-up: Prelu/Square/Abs_reciprocal_sqrt all live in the
            # abs_reciprocal_sqrt_and_small set -> table loads overlap the
            # input DMA latency.
            nc.vector.memset(warm[:], 1.0)
            nc.scalar.activation(
                out=warm[:], in_=warm[:], func=Act.Abs_reciprocal_sqrt
            )
            # selector blockdiag(J/64, J/64) and the ones rows, built on chip
            nc.gpsimd.memset(sel[:], 0.0)
            nc.gpsimd.memset(sel[0:64, 0:64], 1.0 / D)
            nc.gpsimd.memset(sel[64:128, 64:128], 1.0 / D)
            nc.gpsimd.memset(ones2[:], 1.0)
            nc.gpsimd.memset(epsb[:], EPS)

            # ---- phase A: MM1 both chunks first, then per-chunk compute ----
            for c in range(NCH):
                nc.tensor.matmul(
                    out=psum1[c][:], lhsT=wblk, rhs=xcol(c),
                    start=True, stop=True,
                )
            for c in range(NCH):
                cs = slice(c * CW, (c + 1) * CW)
                # t = lrelu(pre + v1): parametric relu, alpha = slope
                nc.scalar.activation(
                    out=t_sb[:, cs], in_=psum1[c][:], func=Act.Prelu,
                    bias=v1b, alpha=SLOPE,
                )
                nc.vector.tensor_tensor(
                    out=sq_sb[:, cs], in0=t_sb[:, cs], in1=t_sb[:, cs],
                    op=Alu.mult,
                )
                nc.tensor.matmul(
                    out=pmean[c][:], lhsT=sel[:], rhs=t_sb[:, cs],
                    start=True, stop=True,
                )
                nc.tensor.matmul(
                    out=pmsq[c][:], lhsT=sel[:], rhs=sq_sb[:, cs],
                    start=True, stop=True,
                )
                # mean^2 early so the stt below only waits on MM2b
                nc.scalar.activation(
                    out=mean2[:, cs], in_=pmean[c][:], func=Act.Square,
                )

            # ---- stats: rstd = 1/sqrt(E[t^2] - E[t]^2 + eps), replicated ----
            for c in range(NCH):
                cs = slice(c * CW, (c + 1) * CW)
                nc.vector.scalar_tensor_tensor(
                    out=rstd[:, cs], in0=pmsq[c][:], scalar=1.0,
                    in1=mean2[:, cs], op0=Alu.mult, op1=Alu.subtract,
                )
                # rstd = 1/sqrt(var + eps), single ACT op (var >= 0)
                nc.scalar.activation(
                    out=rstd[:, cs], in_=rstd[:, cs],
                    func=Act.Abs_reciprocal_sqrt, bias=epsb[:],
                )
                nc.vector.tensor_tensor(
                    out=ts_sb[:, cs], in0=t_sb[:, cs], in1=rstd[:, cs],
                    op=Alu.mult,
                )

            # ---- phase C: pair-tiles -- two 128-node tiles share one PSUM
            # [128,128] and one Prelu.  Pair p covers node tiles (2p, 2p+1);
            # pair order follows ts chunk readiness.
            for a in (0, 4, 2, 6):
                h = a // 4
                hp = slice(64 * h, 64 * h + 64)
                pp = pcpool.tile([128, 2 * D], fp32, tag="pp")
                for k in range(2):
                    i = a + k
                    j = i % 4
                    nc.tensor.matmul(
                        out=pp[:, 64 * k:64 * k + 64],
                        lhsT=ts_sb[hp, 128 * j:128 * j + 128],
                        rhs=m2two[hp, :],
                        start=True, stop=False,
                        skip_group_check=True,
                    )
                    nc.tensor.matmul(
                        out=pp[:, 64 * k:64 * k + 64],
                        lhsT=ones2[64 * h:64 * h + 1, 0:128],
                        rhs=v2two[64 * h:64 * h + 1, :],
                        start=False, stop=True,
                        skip_group_check=True,
                    )
                nc.scalar.activation(
                    out=o_all[:, 64 * a:64 * a + 128], in_=pp[:],
                    func=Act.Prelu, alpha=SLOPE,
                )
                # output DMA for this pair (256 contiguous y rows)
                dst = y_d[128 * a:128 * a + 256, :]
                dst = dst.rearrange("(i p) f -> p i f", i=2, p=128)
                src = o_all[:, 64 * a:64 * a + 128].rearrange(
                    "p (i f) -> p i f", i=2, f=D
                )
                nc.sync.dma_start(out=dst, in_=src)

    return nc


def kernel(**inputs):
    if not _edges_degenerate(inputs["edge_src"], inputs["edge_dst"]):
        return _numpy_fallback(inputs)

    import ml_dtypes
    from concourse.bass_utils import run_bass_kernel_spmd

    bf = ml_dtypes.bfloat16
    M1, v1, M2c, v2 = _fold_weights(inputs)

    wblk = np.zeros((128, 128), np.float64)
    wblk[0:64, 0:64] = M1
    wblk[64:128, 64:128] = M1
    m2two = np.vstack([M2c, M2c])                     # [128, 64]
    v2two = np.zeros((128, 64), np.float64)
    v2two[0, :] = v2
    v2two[64, :] = v2
    wpk = np.hstack([wblk, m2two, v2two]).astype(bf)  # [128, 256]
    # v1 duplicated per half, fp32, carried as 2 bf16 columns of xat
    v1b = np.concatenate([v1, v1]).reshape(128, 1).astype(np.float32)
    v1cols = v1b.view(np.uint16).view(bf)             # [128, 2]

    xf = np.asarray(inputs["x"], np.float32).reshape(N, D)
    in_maps = []
    for c in range(NCORES):
        xs = xf[c * RPC:(c + 1) * RPC]                # [1024, 64]
        xat = np.concatenate(
            [xs[0:HALF].T, xs[HALF:RPC].T], axis=0    # [128, 512]
        ).astype(bf)
        xat = np.concatenate([v1cols, xat], axis=1)   # [128, 514]
        in_maps.append({
            "xat": np.ascontiguousarray(xat),
            "wpk": wpk,
        })

    nc = build_bass()
    if not nc.is_finalized():
        nc.finalize()
    res = run_bass_kernel_spmd(nc, in_maps, list(range(NCORES)))
    global LAST_RESULT
    LAST_RESULT = res
    out = np.concatenate([r["y"] for r in res.results], 0)
    return out.reshape(B, W, D).astype(np.float32)


LAST_RESULT = None


if __name__ == "__main__":
    print("kernel module ok")


# revision 12
# speedup vs baseline: 2.2566x; 1.0421x over previous
"""Trainium2 Bass kernel for nn_HSR_2_25116968747549 (gnn_message_passing).

The reference's edge construction (`tile(B,1).reshape(2,-1)`, the preserved
index-mixing bug) makes `edge_src == edge_dst` for every edge: all edges are
self-edges.  For a segment whose edges all share src == dst == n,
    out[n] = sum_e alpha_e * xl[src_e] = xl[n] * sum_e alpha_e = xl[n]
regardless of the attention logits, so each GATv2 layer collapses to the dense
affine map  x -> (x @ Wl + bl + cb) @ linw  and Wr/br/att never affect the
output.  The whole network is then

    t   = leaky_relu(x @ M1 + v1, 0.01)          M1 = Wl1@linw1@w1  (64x64)
    t_n = layernorm(t) * gamma + beta
    out = leaky_relu(t_n @ M2 + v2, 0.01)        M2 folded likewise

LayerNorm folds further: (t - mu) = t @ C with C = I - J/64, and the per-row
rstd scale commutes past the second matmul, so on device:

    t    = lrelu(x @ M1 + v1)
    a_r  = rsqrt(mean(t^2) - mean(t)^2 + eps)
    out  = lrelu((a_r * t) @ M2c + v2)           M2c = C @ diag(gamma) @ M2

Device layout (per core, 1024 nodes): "two-half transposed" [128, 512]:
partitions 0-63 hold the 64 features of nodes 0-511 (one node per column),
partitions 64-127 hold nodes 512-1023.  This makes every matmul
weight-stationary with zero on-chip transposes:

  MM1:   psum1 = blockdiag(M1,M1)^T @ xT2            [128, 512]
  lrelu: t = ACT(Lrelu, psum1, bias=[v1;v1], alpha=0.01)  (scalar engine)
  MM2:   mean/meansq replicated per half via blockdiag(J/64,J/64) stationary
  stats: var = msq - mean^2; sd = sqrt(var+eps); rstd = 1/sd; ts = t*rstd
  MM3:   out tile i = (ts slice [64,128])^T @ M2c + ones^T v2   [128, 64]
  ACT(Lrelu) -> o_all -> 4 rearranged DMAs to row-major y.

All matmuls run in bf16 (1 cycle/row vs 4 for fp32; tolerance is 2e-2 and
bf16 lands ~1e-3).  Weights fold on host in fp64.
"""

import numpy as np

B, W, D, H = 256, 32, 64, 4
N = B * W
NCORES = 8
RPC = N // NCORES          # rows (nodes) per core = 1024
HALF = RPC // 2            # 512 nodes per half
EPS = 1e-5
SLOPE = 0.01


def _fold_weights(inp):
    f = lambda k: np.asarray(inp[k], np.float64)
    M1 = f("Wl1") @ f("linw1") @ f("w1")
    v1 = (f("bl1") + f("cb1")) @ f("linw1") @ f("w1") + f("b1")
    A2w = f("Wl2") @ f("linw2") @ f("w2")
    M2 = f("gamma")[:, None] * A2w
    v2 = f("beta") @ A2w + (f("bl2") + f("cb2")) @ f("linw2") @ f("w2") + f("b2")
    Cm = np.eye(D) - 1.0 / D
    M2c = Cm @ M2
    return M1, v1, M2c, v2


def _edges_degenerate(src, dst):
    src = np.asarray(src)
    dst = np.asarray(dst)
    return src.shape == dst.shape and np.array_equal(src, dst) and np.all(
        np.bincount(dst.astype(np.int64), minlength=N)[:N] > 0
    )


def _numpy_fallback(inp):
    # Generic (slow) host implementation, only used if the edge arrays ever
    # stop being fully degenerate.
    x = np.asarray(inp["x"], np.float32).reshape(N, D)
    src = np.asarray(inp["edge_src"]).astype(np.int64)
    dst = np.asarray(inp["edge_dst"]).astype(np.int64)

    def gat(xf, Wl, bl, Wr, br, att, cb, linw):
        xl = (xf @ Wl + bl).reshape(N, H, D)
        xr = (xf @ Wr + br).reshape(N, H, D)
        e = xl[src] + xr[dst]
        e = np.where(e > 0, e, 0.2 * e)
        logits = np.einsum("ehd,hd->eh", e, att)
        m = np.full((N, H), -np.inf, np.float32)
        np.maximum.at(m, dst, logits)
        ex = np.exp(logits - m[dst])
        den = np.zeros((N, H), np.float32)
        np.add.at(den, dst, ex)
        alpha = ex / den[dst]
        out = np.zeros((N, H, D), np.float32)
        np.add.at(out, dst, xl[src] * alpha[:, :, None])
        return (out.reshape(N, H * D) + cb) @ linw

    g = lambda k: np.asarray(inp[k], np.float32)
    lr = lambda t, a: np.where(t > 0, t, a * t)
    out = gat(x, g("Wl1"), g("bl1"), g("Wr1"), g("br1"), g("att1"), g("cb1"), g("linw1"))
    out = lr(out @ g("w1") + g("b1"), 0.01)
    mu = out.mean(-1, keepdims=True)
    var = ((out - mu) ** 2).mean(-1, keepdims=True)
    out = (out - mu) / np.sqrt(var + EPS) * g("gamma") + g("beta")
    out = gat(out, g("Wl2"), g("bl2"), g("Wr2"), g("br2"), g("att2"), g("cb2"), g("linw2"))
    out = lr(out @ g("w2") + g("b2"), 0.01)
    return out.reshape(B, W, D).astype(np.float32)


def build_bass():
    from concourse import bacc, mybir
    import concourse.tile as tile

    fp32 = mybir.dt.float32
    bf16 = mybir.dt.bfloat16
    Act = mybir.ActivationFunctionType
    Alu = mybir.AluOpType

    nc = bacc.Bacc()
    # cols 0-1: v1 (fp32 bitcast as 2 bf16 cols); cols 2-513: x two-half layout
    xat_d = nc.declare_dram_parameter("xat", [128, HALF + 2], bf16, isOutput=False)
    wpk_d = nc.declare_dram_parameter("wpk", [128, 320], bf16, isOutput=False)
    y_d = nc.declare_dram_parameter("y", [RPC, D], fp32, isOutput=True)

    with tile.TileContext(nc) as tc:
        with (
            tc.tile_pool(name="const", bufs=1) as cpool,
            tc.tile_pool(name="psum", bufs=1, space="PSUM") as ppool,
            tc.tile_pool(name="pc", bufs=2, space="PSUM") as pcpool,
        ):
            # ---- persistent tiles ----
            xat = cpool.tile([128, HALF + 2], bf16, tag="xat")
            wpk = cpool.tile([128, 320], bf16, tag="wpk")
            sel = cpool.tile([128, 128], bf16, tag="sel")
            ones2 = cpool.tile([128, 128], bf16, tag="ones2")
            epsb = cpool.tile([128, 1], fp32, tag="epsb")
            warm = cpool.tile([1, 1], fp32, tag="warm")
            t_sb = cpool.tile([128, HALF], bf16, tag="t_sb")
            sq_sb = cpool.tile([128, HALF], bf16, tag="sq_sb")
            mean2 = cpool.tile([128, HALF], fp32, tag="mean2")
            rstd = cpool.tile([128, HALF], fp32, tag="rstd")
            ts_sb = cpool.tile([128, HALF], bf16, tag="ts_sb")
            o_all = cpool.tile([128, HALF], fp32, tag="o_all")

            NCH = 2
            CW = HALF // NCH  # 256-column chunks
            psum1 = [ppool.tile([128, CW], fp32, name=f"psum1_{c}", tag=f"psum1_{c}") for c in range(NCH)]
            pmean = [ppool.tile([128, CW], fp32, name=f"pmean_{c}", tag=f"pmean_{c}") for c in range(NCH)]
            pmsq = [ppool.tile([128, CW], fp32, name=f"pmsq_{c}", tag=f"pmsq_{c}") for c in range(NCH)]

            wblk = wpk[:, 0:128]          # blockdiag(M1, M1)
            m2two = wpk[:, 128:192]       # M2c stacked twice (rows 0-63 / 64-127)
            v2pair = wpk[:, 192:320]      # [v2||v2] in rows 0 and 64
            v1b = xat[:, 0:2].bitcast(fp32)  # [128, 1] fp32 view

            def xcol(c):  # data columns of chunk c (skipping the v1 prefix)
                return xat[:, 2 + c * CW:2 + (c + 1) * CW]

            # ---- input DMAs + constants (first thing on each queue) ----
            for c in range(NCH):
                lo = 0 if c == 0 else 2 + c * CW
                hi = 2 + (c + 1) * CW
                nc.sync.dma_start(out=xat[:, lo:hi], in_=xat_d[:, lo:hi])
            nc.scalar.dma_start(out=wpk[:], in_=wpk_d[:])
            # ACT warm-up: Prelu/Square/Abs_reciprocal_sqrt all live in the
            # abs_reciprocal_sqrt_and_small set -> table loads overlap the
            # input DMA latency.
            nc.vector.memset(warm[:], 1.0)
            nc.scalar.activation(
                out=warm[:], in_=warm[:], func=Act.Abs_reciprocal_sqrt
            )
            # selector blockdiag(J/64, J/64) and the ones rows, built on chip
            nc.gpsimd.memset(sel[:], 0.0)
            nc.gpsimd.memset(sel[0:64, 0:64], 1.0 / D)
            nc.gpsimd.memset(sel[64:128, 64:128], 1.0 / D)
            nc.gpsimd.memset(ones2[:], 1.0)
            nc.gpsimd.memset(epsb[:], EPS)

            # ---- phase A: MM1 both chunks first, then per-chunk compute ----
            for c in range(NCH):
                nc.tensor.matmul(
                    out=psum1[c][:], lhsT=wblk, rhs=xcol(c),
                    start=True, stop=True,
                )
            for c in range(NCH):
                cs = slice(c * CW, (c + 1) * CW)
                # t = lrelu(pre + v1): parametric relu, alpha = slope
                nc.scalar.activation(
                    out=t_sb[:, cs], in_=psum1[c][:], func=Act.Prelu,
                    bias=v1b, alpha=SLOPE,
                )
                nc.vector.tensor_tensor(
                    out=sq_sb[:, cs], in0=t_sb[:, cs], in1=t_sb[:, cs],
                    op=Alu.mult,
                )
                nc.tensor.matmul(
                    out=pmean[c][:], lhsT=sel[:], rhs=t_sb[:, cs],
                    start=True, stop=True,
                )
                nc.tensor.matmul(
                    out=pmsq[c][:], lhsT=sel[:], rhs=sq_sb[:, cs],
                    start=True, stop=True,
                )
                # mean^2 early so the stt below only waits on MM2b
                nc.scalar.activation(
                    out=mean2[:, cs], in_=pmean[c][:], func=Act.Square,
                )

            # ---- stats: rstd = 1/sqrt(E[t^2] - E[t]^2 + eps), replicated ----
            for c in range(NCH):
                cs = slice(c * CW, (c + 1) * CW)
                nc.vector.scalar_tensor_tensor(
                    out=rstd[:, cs], in0=pmsq[c][:], scalar=1.0,
                    in1=mean2[:, cs], op0=Alu.mult, op1=Alu.subtract,
                )
                # rstd = 1/sqrt(var + eps), single ACT op (var >= 0)
                nc.scalar.activation(
                    out=rstd[:, cs], in_=rstd[:, cs],
                    func=Act.Abs_reciprocal_sqrt, bias=epsb[:],
                )
                nc.vector.tensor_tensor(
                    out=ts_sb[:, cs], in0=t_sb[:, cs], in1=rstd[:, cs],
                    op=Alu.mult,
                )

            # ---- phase C: pair-tiles -- two 128-node tiles share one PSUM
            # [128,128] and one Prelu.  Pair p covers node tiles (2p, 2p+1);
            # pair order follows ts chunk readiness.
            for a in (0, 4, 2, 6):
                h = a // 4
                hp = slice(64 * h, 64 * h + 64)
                pp = pcpool.tile([128, 2 * D], fp32, tag="pp")
                nc.tensor.matmul(
                    out=pp[:],
                    lhsT=ones2[64 * h:64 * h + 1, 0:128],
                    rhs=v2pair[64 * h:64 * h + 1, :],
                    start=True, stop=False,
                    skip_group_check=True,
                )
                for k in range(2):
                    j = (a + k) % 4
                    nc.tensor.matmul(
                        out=pp[:, 64 * k:64 * k + 64],
                        lhsT=ts_sb[hp, 128 * j:128 * j + 128],
                        rhs=m2two[hp, :],
                        start=False, stop=(k == 1),
                        skip_group_check=True,
                    )
                nc.scalar.activation(
                    out=o_all[:, 64 * a:64 * a + 128], in_=pp[:],
                    func=Act.Prelu, alpha=SLOPE,
                )
                # output DMA for this pair (256 contiguous y rows); the last
                # pair issues from the scalar engine right after its Prelu.
                dst = y_d[128 * a:128 * a + 256, :]
                dst = dst.rearrange("(i p) f -> p i f", i=2, p=128)
                src = o_all[:, 64 * a:64 * a + 128].rearrange(
                    "p (i f) -> p i f", i=2, f=D
                )
                eng = nc.scalar if a == 6 else nc.sync
                eng.dma_start(out=dst, in_=src)

    return nc


def kernel(**inputs):
    if not _edges_degenerate(inputs["edge_src"], inputs["edge_dst"]):
        return _numpy_fallback(inputs)

    import ml_dtypes
    from concourse.bass_utils import run_bass_kernel_spmd

    bf = ml_dtypes.bfloat16
    M1, v1, M2c, v2 = _fold_weights(inputs)

    wblk = np.zeros((128, 128), np.float64)
    wblk[0:64, 0:64] = M1
    wblk[64:128, 64:128] = M1
    m2two = np.vstack([M2c, M2c])                     # [128, 64]
    v2pair = np.zeros((128, 128), np.float64)
    v2pair[0, :] = np.concatenate([v2, v2])
    v2pair[64, :] = np.concatenate([v2, v2])
    wpk = np.hstack([wblk, m2two, v2pair]).astype(bf)  # [128, 320]
    # v1 duplicated per half, fp32, carried as 2 bf16 columns of xat
    v1b = np.concatenate([v1, v1]).reshape(128, 1).astype(np.float32)
    v1cols = v1b.view(np.uint16).view(bf)             # [128, 2]

    xf = np.asarray(inputs["x"], np.float32).reshape(N, D)
    in_maps = []
    for c in range(NCORES):
        xs = xf[c * RPC:(c + 1) * RPC]                # [1024, 64]
        xat = np.concatenate(
            [xs[0:HALF].T, xs[HALF:RPC].T], axis=0    # [128, 512]
        ).astype(bf)
        xat = np.concatenate([v1cols, xat], axis=1)   # [128, 514]
        in_maps.append({
            "xat": np.ascontiguousarray(xat),
            "wpk": wpk,
        })

    nc = build_bass()
    if not nc.is_finalized():
        nc.finalize()
    res = run_bass_kernel_spmd(nc, in_maps, list(range(NCORES)))
    global LAST_RESULT
    LAST_RESULT = res
    out = np.concatenate([r["y"] for r in res.results], 0)
    return out.reshape(B, W, D).astype(np.float32)


LAST_RESULT = None


if __name__ == "__main__":
    print("kernel module ok")
